# revision 1
# baseline (speedup 1.0000x reference)
"""Trainium2 Bass kernel: bidirectional GNN message passing (scatter-add) + concat.

Computation (per batch b):
    out[b, :, 0:256]   = M_b @ x[b]        where M_b[i, j] = (# edges i<-j) + (# edges j<-i)
    out[b, :, 256:512] = x[b]

M_b is a symmetric count matrix built on the host from the edge indices (pure
index preprocessing; all x-dependent arithmetic runs on the NeuronCores).
Sharding: data-parallel over the batch dim, 4 batches per core on 8 cores.
On-device the scatter-add is computed as dense 128x128-block matmuls on the
tensor engine (fp32r, full rate at N=256) with PSUM accumulation over the 16
source-node blocks; the adjacency streams from HBM as uint8 and is cast
u8->f32r on the vector/scalar engines.
"""

import numpy as np

B, N, D = 32, 2048, 256
NC = 8                  # cores
BPC = B // NC           # batches per core = 4
NB = N // 128           # node blocks per batch = 16
G = BPC * NB            # node blocks per core = 64

_compiled = None


def _build_bass():
    from contextlib import ExitStack
    import concourse.bass as bass
    import concourse.tile as tile
    from concourse import bacc, mybir

    nc = bacc.Bacc("TRN2", target_bir_lowering=False, debug=False, num_devices=NC)
    x_ap = nc.dram_tensor("x", [BPC * N, D], mybir.dt.float32, kind="ExternalInput").ap()
    a_ap = nc.dram_tensor("a", [BPC, NB, NB, 128, 128], mybir.dt.uint8, kind="ExternalInput").ap()
    out_ap = nc.dram_tensor("out", [BPC * N, 2 * D], mybir.dt.float32, kind="ExternalOutput").ap()

    with tile.TileContext(nc) as tc:
        with ExitStack() as ctx:
            xpool = ctx.enter_context(tc.tile_pool(name="x", bufs=1))
            xrpool = ctx.enter_context(tc.tile_pool(name="xr", bufs=1))
            apool = ctx.enter_context(tc.tile_pool(name="a8", bufs=3))
            afpool = ctx.enter_context(tc.tile_pool(name="af", bufs=3))
            pspool = ctx.enter_context(tc.tile_pool(name="ps", bufs=4, space="PSUM"))
            opool = ctx.enter_context(tc.tile_pool(name="o", bufs=3))

            # x resident in SBUF: [p, (g, d)] where node n = g*128 + p
            x_sb = xpool.tile([128, G * D], mybir.dt.float32)
            nc.sync.dma_start(x_sb[:], x_ap.rearrange("(g p) d -> p g d", p=128))
            # fp32r-rounded copy for the PE (split across DVE and ACT)
            x_r = xrpool.tile([128, G * D], mybir.dt.float32r)
            half = G * D // 2
            nc.vector.tensor_copy(x_r[:, :half], x_sb[:, :half])
            nc.scalar.copy(x_r[:, half:], x_sb[:, half:])

            for b in range(BPC):
                for i in range(NB):
                    g = b * NB + i
                    a_u8 = apool.tile([128, NB * 128], mybir.dt.uint8)
                    nc.sync.dma_start(a_u8[:], a_ap[b, i].rearrange("j s d -> s j d"))
                    a_f = afpool.tile([128, NB * 128], mybir.dt.float32r)
                    nc.vector.tensor_copy(a_f[:, : NB * 64], a_u8[:, : NB * 64])
                    nc.scalar.copy(a_f[:, NB * 64 :], a_u8[:, NB * 64 :])
                    pt = pspool.tile([128, D], mybir.dt.float32)
                    for j in range(NB):
                        nc.tensor.matmul(
                            pt[:],
                            a_f[:, j * 128 : (j + 1) * 128],
                            x_r[:, (b * NB + j) * D : (b * NB + j + 1) * D],
                            start=(j == 0),
                            stop=(j == NB - 1),
                        )
                    o_t = opool.tile([128, 2 * D], mybir.dt.float32)
                    nc.vector.tensor_copy(o_t[:, :D], pt[:])
                    nc.scalar.copy(o_t[:, D:], x_sb[:, g * D : (g + 1) * D])
                    nc.sync.dma_start(out_ap[g * 128 : (g + 1) * 128, :], o_t[:])

    nc.compile()
    return nc


def _host_build_adjacency(batch_idx, src_idx, dst_idx):
    """Per-batch symmetric count matrices, laid out as lhsT blocks.

    Returns uint8 array [B, NB, NB, 128, 128]: a[b, i, j, s, d] =
    M_b[j*128+s, i*128+d] (M symmetric, so this is the [src, dst] block
    feeding dst-block i from src-block j).
    """
    a = np.empty((B, NB, NB, 128, 128), dtype=np.uint8)
    order = np.argsort(batch_idx, kind="stable")
    bcounts = np.bincount(batch_idx.astype(np.int64), minlength=B)
    offs = np.zeros(B + 1, dtype=np.int64)
    np.cumsum(bcounts, out=offs[1:])
    src_s = src_idx[order].astype(np.int64)
    dst_s = dst_idx[order].astype(np.int64)
    for b in range(B):
        s = src_s[offs[b] : offs[b + 1]]
        d = dst_s[offs[b] : offs[b + 1]]
        ids = np.concatenate([d * N + s, s * N + d])
        m = np.bincount(ids, minlength=N * N)
        # m[row, col]: row = src (lhsT partition), col = dst (because M symmetric)
        mr = m.reshape(NB, 128, NB, 128)  # [J, s, I, d]
        a[b] = mr.transpose(2, 0, 1, 3).astype(np.uint8)  # [I, J, s, d]
    return a


def kernel(x, batch_idx, src_idx, dst_idx):
    global _compiled
    from concourse import bass_utils

    assert x.shape == (B, N, D), x.shape
    a_all = _host_build_adjacency(batch_idx, src_idx, dst_idx)

    if _compiled is None:
        _compiled = _build_bass()
    nc = _compiled

    in_maps = []
    for c in range(NC):
        xs = np.ascontiguousarray(
            x[c * BPC : (c + 1) * BPC].reshape(BPC * N, D).astype(np.float32)
        )
        asrd = np.ascontiguousarray(a_all[c * BPC : (c + 1) * BPC])
        in_maps.append({"x": xs, "a": asrd})

    res = bass_utils.run_bass_kernel_spmd(nc, in_maps, core_ids=list(range(NC)))

    out = np.empty((B, N, 2 * D), dtype=np.float32)
    for c in range(NC):
        out[c * BPC : (c + 1) * BPC] = res.results[c]["out"].reshape(BPC, N, 2 * D)
    return out


# revision 5
# speedup vs baseline: 1.2229x; 1.2229x over previous
"""Trainium2 Bass kernel: bidirectional GNN message passing (scatter-add) + concat.

Computation (per batch b):
    out[b, :, 0:256]   = M_b @ x[b]        where M_b[i, j] = (# edges i<-j) + (# edges j<-i)
    out[b, :, 256:512] = x[b]

M_b is a symmetric count matrix built on the host from the edge indices (pure
index preprocessing; all x-dependent arithmetic runs on the NeuronCores).
Sharding: data-parallel over the batch dim, 4 batches per core on 8 cores.
On-device the scatter-add is computed as dense 128x128-block matmuls on the
tensor engine (fp32r, full rate at N=256) with PSUM accumulation over the 16
source-node blocks; the adjacency streams from HBM as uint8 and is cast
u8->f32r on the vector/scalar engines.
"""

import numpy as np

B, N, D = 32, 2048, 256
NC = 8                  # cores
BPC = B // NC           # batches per core = 4
NB = N // 128           # node blocks per batch = 16
G = BPC * NB            # node blocks per core = 64

_compiled = None


def _build_bass():
    from contextlib import ExitStack
    import concourse.bass as bass
    import concourse.tile as tile
    from concourse import bacc, mybir

    nc = bacc.Bacc("TRN2", target_bir_lowering=False, debug=False, num_devices=NC)
    x_ap = nc.dram_tensor("x", [BPC * N, D], mybir.dt.float32, kind="ExternalInput").ap()
    # A layout [b, I, s, J, d]: per dst-strip I, partition row s is contiguous
    # (NB*128 = 2KB) in HBM so the strip DMA uses full-line descriptors.
    a_ap = nc.dram_tensor("a", [BPC, NB, 128, NB, 128], mybir.dt.uint8, kind="ExternalInput").ap()
    out_ap = nc.dram_tensor("out", [BPC * N, 2 * D], mybir.dt.float32, kind="ExternalOutput").ap()

    with tile.TileContext(nc) as tc:
        with ExitStack() as ctx:
            xpool = ctx.enter_context(tc.tile_pool(name="x", bufs=1))
            xrpool = ctx.enter_context(tc.tile_pool(name="xr", bufs=1))
            apool = ctx.enter_context(tc.tile_pool(name="a8", bufs=3))
            afpool = ctx.enter_context(tc.tile_pool(name="af", bufs=3))
            pspool = ctx.enter_context(tc.tile_pool(name="ps", bufs=4, space="PSUM"))
            opool = ctx.enter_context(tc.tile_pool(name="o", bufs=3))

            # x resident in SBUF: [p, (g, d)] where node n = g*128 + p
            x_sb = xpool.tile([128, G * D], mybir.dt.float32)
            nc.sync.dma_start(x_sb[:], x_ap.rearrange("(g p) d -> p g d", p=128))
            # fp32r-rounded copy for the PE (split across DVE and ACT)
            x_r = xrpool.tile([128, G * D], mybir.dt.float32r)
            half = G * D // 2
            nc.vector.tensor_copy(x_r[:, :half], x_sb[:, :half])
            nc.scalar.copy(x_r[:, half:], x_sb[:, half:])

            for b in range(BPC):
                for i in range(NB):
                    g = b * NB + i
                    a_u8 = apool.tile([128, NB * 128], mybir.dt.uint8)
                    nc.sync.dma_start(a_u8[:], a_ap[b, i].rearrange("s j d -> s (j d)"))
                    a_f = afpool.tile([128, NB * 128], mybir.dt.float32r)
                    nc.vector.tensor_copy(a_f[:, : NB * 64], a_u8[:, : NB * 64])
                    nc.scalar.copy(a_f[:, NB * 64 :], a_u8[:, NB * 64 :])
                    pt = pspool.tile([128, D], mybir.dt.float32)
                    for j in range(NB):
                        nc.tensor.matmul(
                            pt[:],
                            a_f[:, j * 128 : (j + 1) * 128],
                            x_r[:, (b * NB + j) * D : (b * NB + j + 1) * D],
                            start=(j == 0),
                            stop=(j == NB - 1),
                        )
                    o_t = opool.tile([128, 2 * D], mybir.dt.float32)
                    nc.vector.tensor_copy(o_t[:, :D], pt[:])
                    nc.scalar.copy(o_t[:, D:], x_sb[:, g * D : (g + 1) * D])
                    nc.sync.dma_start(out_ap[g * 128 : (g + 1) * 128, :], o_t[:])

    nc.compile()
    return nc


def _host_build_adjacency(batch_idx, src_idx, dst_idx):
    """Per-batch symmetric count matrices, laid out as lhsT blocks.

    Returns uint8 array [B, NB, 128, NB, 128]: a[b, i, s, j, d] =
    M_b[j*128+s, i*128+d] (M symmetric, so this is the [src, dst] block
    feeding dst-block i from src-block j), strip-row-contiguous for DMA.
    """
    a = np.empty((B, NB, 128, NB, 128), dtype=np.uint8)
    order = np.argsort(batch_idx, kind="stable")
    bcounts = np.bincount(batch_idx.astype(np.int64), minlength=B)
    offs = np.zeros(B + 1, dtype=np.int64)
    np.cumsum(bcounts, out=offs[1:])
    src_s = src_idx[order].astype(np.int64)
    dst_s = dst_idx[order].astype(np.int64)
    for b in range(B):
        s = src_s[offs[b] : offs[b + 1]]
        d = dst_s[offs[b] : offs[b + 1]]
        ids = np.concatenate([d * N + s, s * N + d])
        m = np.bincount(ids, minlength=N * N)
        # m[row, col]: row = src (lhsT partition), col = dst (because M symmetric)
        mr = m.reshape(NB, 128, NB, 128)  # [J, s, I, d]
        a[b] = mr.transpose(2, 1, 0, 3).astype(np.uint8)  # [I, s, J, d]
    return a


def kernel(x, batch_idx, src_idx, dst_idx):
    global _compiled
    from concourse import bass_utils

    assert x.shape == (B, N, D), x.shape
    a_all = _host_build_adjacency(batch_idx, src_idx, dst_idx)

    if _compiled is None:
        _compiled = _build_bass()
    nc = _compiled

    in_maps = []
    for c in range(NC):
        xs = np.ascontiguousarray(
            x[c * BPC : (c + 1) * BPC].reshape(BPC * N, D).astype(np.float32)
        )
        asrd = np.ascontiguousarray(a_all[c * BPC : (c + 1) * BPC])
        in_maps.append({"x": xs, "a": asrd})

    res = bass_utils.run_bass_kernel_spmd(nc, in_maps, core_ids=list(range(NC)))

    out = np.empty((B, N, 2 * D), dtype=np.float32)
    for c in range(NC):
        out[c * BPC : (c + 1) * BPC] = res.results[c]["out"].reshape(BPC, N, 2 * D)
    return out


# revision 6
# speedup vs baseline: 1.2527x; 1.0244x over previous
"""Trainium2 Bass kernel: bidirectional GNN message passing (scatter-add) + concat.

Computation (per batch b):
    out[b, :, 0:256]   = M_b @ x[b]        where M_b[i, j] = (# edges i<-j) + (# edges j<-i)
    out[b, :, 256:512] = x[b]

M_b is a symmetric count matrix built on the host from the edge indices (pure
index preprocessing; all x-dependent arithmetic runs on the NeuronCores).
Sharding: data-parallel over the batch dim, 4 batches per core on 8 cores.
On-device the scatter-add is computed as dense 128x128-block matmuls on the
tensor engine (f16 x f16 -> fp32 PSUM accumulation over the 16 source-node
blocks; counts are exact in f16, x is rounded to f16 on the DVE).
"""

import numpy as np

B, N, D = 32, 2048, 256
NC = 8                  # cores
BPC = B // NC           # batches per core = 4
NB = N // 128           # node blocks per batch = 16
G = BPC * NB            # node blocks per core = 64
AMERGE = 4              # strips per A DMA
OMERGE = 4              # strips per out DMA

_compiled = None


def _build_bass():
    from contextlib import ExitStack
    import concourse.bass as bass
    import concourse.tile as tile
    from concourse import bacc, mybir

    nc = bacc.Bacc("TRN2", target_bir_lowering=False, debug=False, num_devices=NC)
    x_ap = nc.dram_tensor("x", [BPC * N, D], mybir.dt.float32, kind="ExternalInput").ap()
    # A layout [b, I, s, J, d] f16: per dst-strip I, partition row s is
    # contiguous (NB*128*2 = 4KB) in HBM -> full-line DMA descriptors.
    a_ap = nc.dram_tensor("a", [BPC, NB, 128, NB, 128], mybir.dt.float16, kind="ExternalInput").ap()
    out_ap = nc.dram_tensor("out", [BPC * N, 2 * D], mybir.dt.float32, kind="ExternalOutput").ap()

    with tile.TileContext(nc) as tc:
        with ExitStack() as ctx:
            xpool = ctx.enter_context(tc.tile_pool(name="x", bufs=1))
            xhpool = ctx.enter_context(tc.tile_pool(name="xh", bufs=1))
            apool = ctx.enter_context(tc.tile_pool(name="a16", bufs=3))
            pspool = ctx.enter_context(tc.tile_pool(name="ps", bufs=4, space="PSUM"))
            opool = ctx.enter_context(tc.tile_pool(name="o", bufs=3))

            # x resident in SBUF: [p, (g, d)] where node n = g*128 + p
            x_sb = xpool.tile([128, G * D], mybir.dt.float32)
            nc.sync.dma_start(x_sb[:], x_ap.rearrange("(g p) d -> p g d", p=128))
            # f16 copy for the PE (split across DVE and ACT)
            x_h = xhpool.tile([128, G * D], mybir.dt.float16)
            half = G * D // 2
            nc.vector.tensor_copy(x_h[:, :half], x_sb[:, :half])
            nc.scalar.copy(x_h[:, half:], x_sb[:, half:])

            for b in range(BPC):
                for im in range(NB // AMERGE):
                    # one DMA covering AMERGE dst-strips of A
                    a_t = apool.tile([128, AMERGE * NB * 128], mybir.dt.float16)
                    nc.sync.dma_start(
                        a_t[:], a_ap[b, im * AMERGE : (im + 1) * AMERGE].rearrange("i s j d -> s i (j d)")
                    )
                    for ii in range(AMERGE):
                        i = im * AMERGE + ii
                        g = b * NB + i
                        a_strip = a_t[:, ii * NB * 128 : (ii + 1) * NB * 128]
                        pt = pspool.tile([128, D], mybir.dt.float32)
                        for j in range(NB):
                            nc.tensor.matmul(
                                pt[:],
                                a_strip[:, j * 128 : (j + 1) * 128],
                                x_h[:, (b * NB + j) * D : (b * NB + j + 1) * D],
                                start=(j == 0),
                                stop=(j == NB - 1),
                            )
                        if i % OMERGE == 0:
                            o_t = opool.tile([128, OMERGE * 2 * D], mybir.dt.float32)
                        oo = i % OMERGE
                        nc.vector.tensor_copy(o_t[:, oo * 2 * D : oo * 2 * D + D], pt[:])
                        nc.scalar.copy(
                            o_t[:, oo * 2 * D + D : (oo + 1) * 2 * D],
                            x_sb[:, g * D : (g + 1) * D],
                        )
                        if i % OMERGE == OMERGE - 1:
                            g0 = b * NB + i - (OMERGE - 1)
                            nc.scalar.dma_start(
                                out_ap[g0 * 128 : (g0 + OMERGE) * 128, :].rearrange(
                                    "(gg p) c -> p gg c", p=128
                                ),
                                o_t[:],
                            )

    nc.compile()
    return nc


def _host_build_adjacency(batch_idx, src_idx, dst_idx):
    """Per-batch symmetric count matrices, laid out as lhsT blocks.

    Returns f16 array [B, NB, 128, NB, 128]: a[b, i, s, j, d] =
    M_b[j*128+s, i*128+d] (M symmetric, so this is the [src, dst] block
    feeding dst-block i from src-block j), strip-row-contiguous for DMA.
    """
    a = np.empty((B, NB, 128, NB, 128), dtype=np.float16)
    order = np.argsort(batch_idx, kind="stable")
    bcounts = np.bincount(batch_idx.astype(np.int64), minlength=B)
    offs = np.zeros(B + 1, dtype=np.int64)
    np.cumsum(bcounts, out=offs[1:])
    src_s = src_idx[order].astype(np.int64)
    dst_s = dst_idx[order].astype(np.int64)
    for b in range(B):
        s = src_s[offs[b] : offs[b + 1]]
        d = dst_s[offs[b] : offs[b + 1]]
        ids = np.concatenate([d * N + s, s * N + d])
        m = np.bincount(ids, minlength=N * N)
        # m[row, col]: row = src (lhsT partition), col = dst (M symmetric)
        mr = m.reshape(NB, 128, NB, 128)  # [J, s, I, d]
        a[b] = mr.transpose(2, 1, 0, 3).astype(np.float16)  # [I, s, J, d]
    return a


def kernel(x, batch_idx, src_idx, dst_idx):
    global _compiled
    from concourse import bass_utils

    assert x.shape == (B, N, D), x.shape
    a_all = _host_build_adjacency(batch_idx, src_idx, dst_idx)

    if _compiled is None:
        _compiled = _build_bass()
    nc = _compiled

    in_maps = []
    for c in range(NC):
        xs = np.ascontiguousarray(
            x[c * BPC : (c + 1) * BPC].reshape(BPC * N, D).astype(np.float32)
        )
        asrd = np.ascontiguousarray(a_all[c * BPC : (c + 1) * BPC])
        in_maps.append({"x": xs, "a": asrd})

    res = bass_utils.run_bass_kernel_spmd(nc, in_maps, core_ids=list(range(NC)))

    out = np.empty((B, N, 2 * D), dtype=np.float32)
    for c in range(NC):
        out[c * BPC : (c + 1) * BPC] = res.results[c]["out"].reshape(BPC, N, 2 * D)
    return out


# revision 9
# speedup vs baseline: 1.2690x; 1.0130x over previous
"""Trainium2 Bass kernel: bidirectional GNN message passing (scatter-add) + concat.

Computation (per batch b):
    out[b, :, 0:256]   = M_b @ x[b]        where M_b[i, j] = (# edges i<-j) + (# edges j<-i)
    out[b, :, 256:512] = x[b]

M_b is a symmetric count matrix built on the host from the edge indices (pure
index preprocessing; all x-dependent arithmetic runs on the NeuronCores).
Sharding: data-parallel over the batch dim, 4 batches per core on 8 cores.
On-device the scatter-add is computed as dense 128x128-block matmuls on the
tensor engine (f16 x f16 -> fp32 PSUM accumulation over the 16 source-node
blocks; counts are exact in f16, x is rounded to f16 on the DVE).
"""

import numpy as np

B, N, D = 32, 2048, 256
NC = 8                  # cores
BPC = B // NC           # batches per core = 4
NB = N // 128           # node blocks per batch = 16
G = BPC * NB            # node blocks per core = 64
AMERGE = 4              # strips per A DMA
OMERGE = 4              # strips per out DMA

_compiled = None


def _build_bass():
    from contextlib import ExitStack
    import concourse.bass as bass
    import concourse.tile as tile
    from concourse import bacc, mybir

    nc = bacc.Bacc("TRN2", target_bir_lowering=False, debug=False, num_devices=NC)
    x_ap = nc.dram_tensor("x", [BPC * N, D], mybir.dt.float32, kind="ExternalInput").ap()
    # A layout [b, I, s, J, d] u8: per dst-strip I, partition row s is
    # contiguous (NB*128 = 2KB) in HBM -> full-line DMA descriptors.
    a_ap = nc.dram_tensor("a", [BPC, NB, 128, NB, 128], mybir.dt.uint8, kind="ExternalInput").ap()
    out_ap = nc.dram_tensor("out", [BPC * N, 2 * D], mybir.dt.float32, kind="ExternalOutput").ap()

    with tile.TileContext(nc) as tc:
        with ExitStack() as ctx:
            xpool = ctx.enter_context(tc.tile_pool(name="x", bufs=1))
            xhpool = ctx.enter_context(tc.tile_pool(name="xh", bufs=1))
            apool = ctx.enter_context(tc.tile_pool(name="a8", bufs=3))
            afpool = ctx.enter_context(tc.tile_pool(name="af", bufs=3))
            pspool = ctx.enter_context(tc.tile_pool(name="ps", bufs=4, space="PSUM"))
            opool = ctx.enter_context(tc.tile_pool(name="o", bufs=3))

            # x resident in SBUF: [p, (g, d)] where node n = g*128 + p
            x_sb = xpool.tile([128, G * D], mybir.dt.float32)
            nc.sync.dma_start(x_sb[:], x_ap.rearrange("(g p) d -> p g d", p=128))
            # f16 copy for the PE (DVE, chunked so it interleaves with strips)
            x_h = xhpool.tile([128, G * D], mybir.dt.float16)
            for q in range(8):
                qn = G * D // 8
                nc.vector.tensor_copy(x_h[:, q * qn : (q + 1) * qn], x_sb[:, q * qn : (q + 1) * qn])

            for b in range(BPC):
                for im in range(NB // AMERGE):
                    # one DMA covering AMERGE dst-strips of A (u8)
                    a_t = apool.tile([128, AMERGE * NB * 128], mybir.dt.uint8)
                    nc.sync.dma_start(
                        a_t[:], a_ap[b, im * AMERGE : (im + 1) * AMERGE].rearrange("i s j d -> s i (j d)")
                    )
                    for ii in range(AMERGE):
                        i = im * AMERGE + ii
                        g = b * NB + i
                        # ACT casts the whole strip u8 -> f16 in one fast op
                        a_f = afpool.tile([128, NB * 128], mybir.dt.float16)
                        nc.scalar.copy(a_f[:], a_t[:, ii * NB * 128 : (ii + 1) * NB * 128])
                        pt = pspool.tile([128, D], mybir.dt.float32)
                        for j in range(NB):
                            nc.tensor.matmul(
                                pt[:],
                                a_f[:, j * 128 : (j + 1) * 128],
                                x_h[:, (b * NB + j) * D : (b * NB + j + 1) * D],
                                start=(j == 0),
                                stop=(j == NB - 1),
                            )
                        if i % OMERGE == 0:
                            o_t = opool.tile([128, OMERGE * 2 * D], mybir.dt.float32)
                        oo = i % OMERGE
                        nc.vector.tensor_copy(o_t[:, oo * 2 * D : oo * 2 * D + D], pt[:])
                        nc.gpsimd.tensor_copy(
                            o_t[:, oo * 2 * D + D : (oo + 1) * 2 * D],
                            x_sb[:, g * D : (g + 1) * D],
                        )
                        if i % OMERGE == OMERGE - 1:
                            g0 = b * NB + i - (OMERGE - 1)
                            nc.scalar.dma_start(
                                out_ap[g0 * 128 : (g0 + OMERGE) * 128, :].rearrange(
                                    "(gg p) c -> p gg c", p=128
                                ),
                                o_t[:],
                            )

    nc.compile()
    return nc


def _host_build_adjacency(batch_idx, src_idx, dst_idx):
    """Per-batch symmetric count matrices, laid out as lhsT blocks.

    Returns u8 array [B, NB, 128, NB, 128]: a[b, i, s, j, d] =
    M_b[j*128+s, i*128+d] (M symmetric, so this is the [src, dst] block
    feeding dst-block i from src-block j), strip-row-contiguous for DMA.
    """
    a = np.empty((B, NB, 128, NB, 128), dtype=np.uint8)
    order = np.argsort(batch_idx, kind="stable")
    bcounts = np.bincount(batch_idx.astype(np.int64), minlength=B)
    offs = np.zeros(B + 1, dtype=np.int64)
    np.cumsum(bcounts, out=offs[1:])
    src_s = src_idx[order].astype(np.int64)
    dst_s = dst_idx[order].astype(np.int64)
    for b in range(B):
        s = src_s[offs[b] : offs[b + 1]]
        d = dst_s[offs[b] : offs[b + 1]]
        ids = np.concatenate([d * N + s, s * N + d])
        m = np.bincount(ids, minlength=N * N)
        # m[row, col]: row = src (lhsT partition), col = dst (M symmetric)
        mr = m.reshape(NB, 128, NB, 128)  # [J, s, I, d]
        a[b] = mr.transpose(2, 1, 0, 3).astype(np.uint8)  # [I, s, J, d]
    return a


def kernel(x, batch_idx, src_idx, dst_idx):
    global _compiled
    from concourse import bass_utils

    assert x.shape == (B, N, D), x.shape
    a_all = _host_build_adjacency(batch_idx, src_idx, dst_idx)

    if _compiled is None:
        _compiled = _build_bass()
    nc = _compiled

    in_maps = []
    for c in range(NC):
        xs = np.ascontiguousarray(
            x[c * BPC : (c + 1) * BPC].reshape(BPC * N, D).astype(np.float32)
        )
        asrd = np.ascontiguousarray(a_all[c * BPC : (c + 1) * BPC])
        in_maps.append({"x": xs, "a": asrd})

    res = bass_utils.run_bass_kernel_spmd(nc, in_maps, core_ids=list(range(NC)))

    out = np.empty((B, N, 2 * D), dtype=np.float32)
    for c in range(NC):
        out[c * BPC : (c + 1) * BPC] = res.results[c]["out"].reshape(BPC, N, 2 * D)
    return out
